# revision 1
# baseline (speedup 1.0000x reference)
"""IntLoRA-SHIFT fused kernel for Trainium2 (8 NeuronCores, tensor-parallel on out_features).

Math (per reference):
    w_int  = ori_weight_round - zero_point                    [O, I]
    lora   = (aux_R + loraB @ loraA) / where(w_int==0, 1, w_int)
    wu     = delta + lora
    weight = sign(wu) * 2^round(log2|wu|) * w_int
    out    = x @ weight.T + bias

Kernel strategy per core (O sharded 8 x 512):
  Phase A (prep, fp32 exact): per 128-row o-chunk, compute weight in [o, i]
    layout with DVE ops.  round(log2|wu|)+exp2+sign is done exactly with an
    integer bit-trick: q = bitcast((bits(wu) + 0x4AFB0C) & 0xFF800000) ==
    sign(wu)*2^round(log2|wu|).  weight = q * w_int is exactly representable
    in bf16 (w_int integer |.|<=255 -> 8 significand bits).  The safe divide
    uses r = reciprocal_approx_accurate(w_int + (w_int==0)) (~2 ulp).
    weight(bf16) is xbar-transposed SBUF->SBUF into wT [i, o] k-tile layout.
  Phase B: x is pre-cast to bf16 and pre-transposed to k-tile-major layout
    [NK, 128, TOK] on the HOST (halves HBM traffic and avoids on-device
    transposes, which measured as the bottleneck).  The kernel streams
    512-token groups and accumulates out[t,o] on PE: 32 bf16 matmuls per
    128-token tile plus a K=1 fp32 matmul that adds the bias row.  PSUM is
    drained by ScalarE copies and stored with HWDGE DMAs.
    Measured 609 us/iteration on 8 axon NCs (slope method, reps=9 For_i).
"""
import os
import sys

for _p in ("/root/.axon_site", "/root/.axon_site/_ro/trn_rl_repo", "/root/.axon_site/_ro/pypackages", "/opt/trn_rl_repo"):
    if os.path.isdir(_p) and _p not in sys.path:
        sys.path.append(_p)

import numpy as np

import concourse.bacc as bacc
import concourse.mybir as mybir
import concourse.tile as tile
from concourse.bass_utils import run_bass_kernel_spmd

A = mybir.AluOpType
F32 = mybir.dt.float32
BF16 = mybir.dt.bfloat16
I32 = mybir.dt.int32

C_ROUND = 0x004AFB0C                                   # carry threshold: mantissa >= sqrt(2)
EXP_MASK = int(np.uint32(0xFF800000).view(np.int32))   # sign+exponent mask

# full problem config
FULL = dict(tok=8192, i=4096, osh=512, r=4, n_cores=8)
B_, S_, O_ = 4, 2048, 4096


def build(tok, i, osh, r, n_cores, reps=1):
    """Build + compile the per-core kernel (SPMD: same program, sharded data).

    reps>1 wraps the whole body in a hardware For_i loop (for benchmarking:
    one dispatch executes the body `reps` times)."""
    nk = i // 128          # contraction k-tiles
    noc = osh // 128       # o-chunks in prep
    ntt = tok // 128       # token tiles
    nic = i // 512         # 512-wide i-chunks for the BA matmul

    nc = bacc.Bacc("TRN2", target_bir_lowering=False, debug=False,
                   enable_asserts=False, num_devices=n_cores)
    xt_d = nc.dram_tensor("xtr", [nk, 128, tok], BF16, kind="ExternalInput").ap()
    ori_d = nc.dram_tensor("ori", [osh, i], F32, kind="ExternalInput").ap()
    aux_d = nc.dram_tensor("aux", [osh, i], F32, kind="ExternalInput").ap()
    zp_d = nc.dram_tensor("zp", [osh, 1], F32, kind="ExternalInput").ap()
    dl_d = nc.dram_tensor("dl", [osh, 1], F32, kind="ExternalInput").ap()
    bt_d = nc.dram_tensor("bt", [r, osh], F32, kind="ExternalInput").ap()   # loraB shard, pre-transposed
    la_d = nc.dram_tensor("la", [r, i], F32, kind="ExternalInput").ap()     # loraA
    bias_d = nc.dram_tensor("bias", [1, osh], F32, kind="ExternalInput").ap()
    out_d = nc.dram_tensor("out", [tok, osh], F32, kind="ExternalOutput").ap()

    import contextlib

    with tile.TileContext(nc) as tc:
        with tc.tile_pool(name="const", bufs=1) as cp, \
             tc.tile_pool(name="wres", bufs=1) as wp, \
             tc.tile_pool(name="prep", bufs=1) as pr, \
             tc.tile_pool(name="bpool", bufs=1) as bp, \
             tc.tile_pool(name="pba", bufs=2, space="PSUM") as pba, \
             tc.tile_pool(name="pps", bufs=4, space="PSUM") as pps, \
             (tc.For_i(0, reps, 1) if reps > 1 else contextlib.nullcontext()):

            # ---- constants
            la_sb = cp.tile([r, i], F32)
            nc.sync.dma_start(la_sb[:], la_d[:])
            bt_sb = cp.tile([r, osh], F32)
            nc.sync.dma_start(bt_sb[:], bt_d[:])
            bias_sb = cp.tile([1, osh], F32)
            nc.sync.dma_start(bias_sb[:], bias_d[:])
            ones_sb = cp.tile([1, 128], F32)
            nc.vector.memset(ones_sb[:], 1.0)

            # resident transposed weight: [i(128), k, o]
            wT = wp.tile([128, nk, osh], BF16)

            # ---- Phase A: weight prep per (o-chunk, i-half)
            hw = min(i, 2048)            # i-half width (SBUF footprint)
            nih = i // hw
            nic_h = hw // 512
            for oc in range(noc):
                osl = slice(oc * 128, (oc + 1) * 128)
                zp_c = pr.tile([128, 1], F32, tag="zp")
                nc.sync.dma_start(zp_c[:], zp_d[osl, :])
                dl_c = pr.tile([128, 1], F32, tag="dl")
                nc.sync.dma_start(dl_c[:], dl_d[osl, :])
                for ih in range(nih):
                    hsl = slice(ih * hw, (ih + 1) * hw)
                    ori_c = pr.tile([128, hw], F32, tag="ori")
                    nc.sync.dma_start(ori_c[:], ori_d[osl, hsl])
                    aux_c = pr.tile([128, hw], F32, tag="aux")
                    nc.sync.dma_start(aux_c[:], aux_d[osl, hsl])

                    # w_int = ori - zp   (exact)
                    wint = pr.tile([128, hw], F32, tag="wint")
                    nc.vector.tensor_scalar(wint[:], ori_c[:], zp_c[:], None, A.subtract)
                    # denom = w_int + (w_int == 0)   -> reuse ori_c slot
                    nc.vector.scalar_tensor_tensor(ori_c[:], wint[:], 0.0, wint[:], A.is_equal, A.add)
                    # r = 1/denom  (~2 ulp)
                    rcp = pr.tile([128, hw], F32, tag="rcp")
                    scr = pr.tile([128, hw], F32, tag="scr")
                    nc.vector.reciprocal_approx_accurate(rcp[:], ori_c[:], scr[:])

                    # num = aux + loraB @ loraA   (PE, fp32, K=r)
                    for ic in range(nic_h):
                        isl = slice(ih * hw + ic * 512, ih * hw + (ic + 1) * 512)
                        lsl = slice(ic * 512, (ic + 1) * 512)
                        ps_ba = pba.tile([128, 512], F32, tag="ba")
                        nc.tensor.matmul(ps_ba[:], bt_sb[:, osl], la_sb[:, isl],
                                         start=True, stop=True)
                        nc.vector.tensor_tensor(aux_c[:, lsl], aux_c[:, lsl], ps_ba[:], A.add)

                    # wu = num * r + delta
                    nc.vector.tensor_tensor(aux_c[:], aux_c[:], rcp[:], A.mult)
                    nc.vector.tensor_scalar(aux_c[:], aux_c[:], dl_c[:], None, A.add)
                    # q = sign(wu) * 2^round(log2|wu|)  (bit trick) -> rcp slot
                    nc.vector.tensor_scalar(rcp[:].bitcast(I32), aux_c[:].bitcast(I32),
                                            C_ROUND, None, A.add)
                    nc.vector.tensor_scalar(rcp[:].bitcast(I32), rcp[:].bitcast(I32),
                                            EXP_MASK, None, A.bitwise_and)
                    # weight = q * w_int  (exact in bf16)
                    wbf = pr.tile([128, hw], BF16, tag="wbf")
                    nc.vector.tensor_tensor(wbf[:], rcp[:], wint[:], A.mult)
                    # transpose into wT[:, k-range of this i-half, o-chunk]
                    nc.sync.dma_start_transpose(wT[:, ih * (hw // 128):(ih + 1) * (hw // 128), osl], wbf[:])

            # ---- Phase B: stream host-transposed bf16 x, matmul, bias, store
            TG = 512
            for tg in range(tok // TG):
                xt = bp.tile([128, nk, TG], BF16, tag="xt", bufs=2)
                nc.sync.dma_start(xt[:], xt_d[:, :, tg * TG:(tg + 1) * TG].rearrange("k p t -> p k t"))
                for ts in range(TG // 128):
                    tt = tg * (TG // 128) + ts
                    ps = pps.tile([128, osh], F32, tag="ps")
                    nc.tensor.matmul(ps[:], ones_sb[:, :], bias_sb[:, :],
                                     start=True, stop=False)      # + bias (K=1, fp32)
                    for k in range(nk):
                        nc.tensor.matmul(ps[:], xt[:, k, ts * 128:(ts + 1) * 128], wT[:, k, :],
                                         start=False, stop=(k == nk - 1))
                    os_t = bp.tile([128, osh], F32, tag="os", bufs=3)
                    nc.scalar.copy(os_t[:], ps[:])
                    nc.scalar.dma_start(out_d[tt * 128:(tt + 1) * 128, :], os_t[:])

    nc.compile()
    return nc


_CACHE = {}


def _get(cfg_key):
    if cfg_key not in _CACHE:
        _CACHE[cfg_key] = build(**dict(cfg_key))
    return _CACHE[cfg_key]


def make_in_maps(x2d, ori, delta, zp, aux, laA, laB, bias, n_cores, osh):
    import ml_dtypes
    nk = x2d.shape[1] // 128
    xtr = np.ascontiguousarray(x2d.astype(ml_dtypes.bfloat16).T).reshape(nk, 128, x2d.shape[0])
    in_maps = []
    for c in range(n_cores):
        sl = slice(c * osh, (c + 1) * osh)
        in_maps.append({
            "xtr": xtr,
            "ori": np.ascontiguousarray(ori[sl]),
            "aux": np.ascontiguousarray(aux[sl]),
            "zp": np.ascontiguousarray(zp[sl]).reshape(osh, 1),
            "dl": np.ascontiguousarray(delta[sl]).reshape(osh, 1),
            "bt": np.ascontiguousarray(laB[sl].T),
            "la": laA,
            "bias": np.ascontiguousarray(bias[sl]).reshape(1, osh),
        })
    return in_maps


def kernel(x, ori_weight_round, weight_quant_delta, weight_quant_zero_point,
           aux_R, loraA_w, loraB_w, bias, _trace=False):
    cfg = FULL
    n_cores, osh = cfg["n_cores"], cfg["osh"]
    x2d = np.ascontiguousarray(np.asarray(x, dtype=np.float32).reshape(cfg["tok"], cfg["i"]))
    nc = _get(tuple(sorted(cfg.items())))
    in_maps = make_in_maps(
        x2d,
        np.asarray(ori_weight_round, np.float32),
        np.asarray(weight_quant_delta, np.float32),
        np.asarray(weight_quant_zero_point, np.float32),
        np.asarray(aux_R, np.float32),
        np.asarray(loraA_w, np.float32),
        np.asarray(loraB_w, np.float32),
        np.asarray(bias, np.float32),
        n_cores, osh)
    res = run_bass_kernel_spmd(nc, in_maps, core_ids=list(range(n_cores)), trace=_trace)
    out = np.concatenate([res.results[c]["out"] for c in range(n_cores)], axis=1)
    out = out.reshape(B_, S_, O_)
    if _trace:
        return out, res
    return out



# revision 10
# speedup vs baseline: 1.2416x; 1.2416x over previous
"""IntLoRA-SHIFT fused kernel for Trainium2 (8 NeuronCores, tensor-parallel on out_features).

Math (per reference):
    w_int  = ori_weight_round - zero_point                    [O, I]
    lora   = (aux_R + loraB @ loraA) / where(w_int==0, 1, w_int)
    wu     = delta + lora
    weight = sign(wu) * 2^round(log2|wu|) * w_int
    out    = x @ weight.T + bias

Kernel strategy per core (O sharded 8 x 512):
  Everything lives in [i(partition), o(free)] layout so weight prep emits the
  k-tile-transposed weight wT directly (no on-device transposes).  The host
  pre-transposes ori (uint8) and aux (fp16) per core; per-o params
  (zero_point/delta/bias) are partition-broadcast once into [128, osh] tiles.

  Phase A (per i-chunk k, tiles [128, 512]):
    PE:  ba = (loraB@loraA)^T slice  (K=4 fp32 matmul into PSUM)
    GpS: wint = ori - zp_b ;  den = wint + (wint==0)
    DVE: rcp ~= 1/den (fast, 18-bit) ; num = aux + ba ; num *= rcp
    GpS: num += dl_b
    DVE: q = bitcast((bits(num) + 0x4AFB0C) & 0xFF800000)   == sign*2^round(log2|.|)
    DVE: wT[:,k,:] = q * wint  (exact in bf16: |wint|<=255 has 8 significand bits)
  Work is split DVE(4-5 passes)/GpSimd(3 passes) so the two engines pipeline
  across chunks; PE+DMA overlap underneath.

  Phase B: host-transposed bf16 x streamed as [128, nk, 512] groups (contig
  32KB/partition DMAs), 32 bf16 matmuls per 128-token tile accumulate in PSUM,
  drained by DVE adds (+bias broadcast tile) and stored with HWDGE DMAs.
  No fp32 matmuls in the loop: PE stays in bf16 mode at the roofline.
"""
import os
import sys

for _p in ("/root/.axon_site", "/root/.axon_site/_ro/trn_rl_repo", "/root/.axon_site/_ro/pypackages", "/opt/trn_rl_repo"):
    if os.path.isdir(_p) and _p not in sys.path:
        sys.path.append(_p)

import numpy as np

import concourse.bacc as bacc
import concourse.mybir as mybir
import concourse.tile as tile
from concourse.bass_utils import run_bass_kernel_spmd

A = mybir.AluOpType
F32 = mybir.dt.float32
F16 = mybir.dt.float16
BF16 = mybir.dt.bfloat16
I32 = mybir.dt.int32
U8 = mybir.dt.uint8

C_ROUND = 0x004AFB0C                                   # carry threshold: mantissa >= sqrt(2)
EXP_MASK = int(np.uint32(0xFF800000).view(np.int32))   # sign+exponent mask

# full problem config
FULL = dict(tok=8192, i=4096, osh=512, r=4, n_cores=8)
B_, S_, O_ = 4, 2048, 4096
TG = 512


def build(tok, i, osh, r, n_cores, reps=1, do_prep=True, do_mm=True, do_bias=True):
    """Build + compile the per-core kernel (SPMD: same program, sharded data).

    reps>1 wraps the whole body in a hardware For_i loop (for benchmarking:
    one dispatch executes the body `reps` times)."""
    nk = i // 128          # contraction k-tiles
    ntg = tok // TG        # 512-token groups

    nc = bacc.Bacc("TRN2", target_bir_lowering=False, debug=False,
                   enable_asserts=False, num_devices=n_cores)
    xt_d = nc.dram_tensor("xtr", [128, ntg, nk, TG], BF16, kind="ExternalInput").ap()
    ori_d = nc.dram_tensor("ori", [128, nk, osh], U8, kind="ExternalInput").ap()
    aux_d = nc.dram_tensor("aux", [128, nk, osh], F16, kind="ExternalInput").ap()
    zp_d = nc.dram_tensor("zp", [1, osh], F32, kind="ExternalInput").ap()
    dl_d = nc.dram_tensor("dl", [1, osh], F32, kind="ExternalInput").ap()
    bt_d = nc.dram_tensor("bt", [r, osh], F32, kind="ExternalInput").ap()   # loraB shard, pre-transposed
    la_d = nc.dram_tensor("la", [r, i], F32, kind="ExternalInput").ap()     # loraA
    bias_d = nc.dram_tensor("bias", [1, osh], F32, kind="ExternalInput").ap()
    out_d = nc.dram_tensor("out", [tok, osh], F32, kind="ExternalOutput").ap()

    import contextlib

    with tile.TileContext(nc) as tc:
        with tc.tile_pool(name="const", bufs=1) as cp, \
             tc.tile_pool(name="wres", bufs=1) as wp, \
             tc.tile_pool(name="prep", bufs=2) as pr, \
             tc.tile_pool(name="bpool", bufs=1) as bp, \
             tc.tile_pool(name="pba", bufs=2, space="PSUM") as pba, \
             tc.tile_pool(name="pps", bufs=4, space="PSUM") as pps, \
             (tc.For_i(0, reps, 1) if reps > 1 else contextlib.nullcontext()):

            # ---- constants + broadcasts
            la_sb = cp.tile([r, i], F32)
            nc.sync.dma_start(la_sb[:], la_d[:])
            bt_sb = cp.tile([r, osh], F32)
            nc.sync.dma_start(bt_sb[:], bt_d[:])
            zp_row = cp.tile([1, osh], F32)
            nc.sync.dma_start(zp_row[:], zp_d[:])
            dl_row = cp.tile([1, osh], F32)
            nc.sync.dma_start(dl_row[:], dl_d[:])
            bias_row = cp.tile([1, osh], F32)
            nc.sync.dma_start(bias_row[:], bias_d[:])

            zp_b = cp.tile([128, osh], F32)
            nc.gpsimd.partition_broadcast(zp_b[:], zp_row[:])
            dl_b = cp.tile([128, osh], F32)
            nc.gpsimd.partition_broadcast(dl_b[:], dl_row[:])
            bias_b = cp.tile([128, osh], F32)
            nc.gpsimd.partition_broadcast(bias_b[:], bias_row[:])
            # whole ori/aux shards resident in SBUF (16KB + 32KB per partition)
            ori_all = cp.tile([128, nk, osh], U8)
            nc.sync.dma_start(ori_all[:], ori_d[:])
            aux_all = cp.tile([128, nk, osh], F16)
            nc.sync.dma_start(aux_all[:], aux_d[:])

            # resident transposed weight: [i(128), k, o]
            wT = wp.tile([128, nk, osh], BF16)
            if not do_prep:
                nc.vector.memset(wT[:, 0, 0:1], 0.0)   # touch so reads are legal

            # ---- Phase A: weight prep per i-chunk
            for k in range(nk if do_prep else 0):
                ba = pba.tile([128, osh], F32, tag="ba")
                nc.tensor.matmul(ba[:], la_sb[:, k * 128:(k + 1) * 128], bt_sb[:],
                                 start=True, stop=True)
                wint = pr.tile([128, osh], F32, tag="wint")
                nc.gpsimd.tensor_tensor(wint[:], ori_all[:, k, :], zp_b[:], A.subtract)
                den = pr.tile([128, osh], F32, tag="den")
                nc.vector.scalar_tensor_tensor(den[:], wint[:], 0.0, wint[:],
                                               A.is_equal, A.add)
                rcp = pr.tile([128, osh], F32, tag="rcp")
                nc.vector.reciprocal_approx_fast(rcp[:], den[:])
                num = pr.tile([128, osh], F32, tag="num")
                nc.vector.tensor_tensor(num[:], aux_all[:, k, :], ba[:], A.add)
                nc.vector.tensor_tensor(num[:], num[:], rcp[:], A.mult)
                nc.gpsimd.tensor_tensor(num[:], num[:], dl_b[:], A.add)
                q = pr.tile([128, osh], F32, tag="q")
                nc.gpsimd.tensor_scalar(q[:].bitcast(I32), num[:].bitcast(I32),
                                        C_ROUND, None, A.add)
                nc.vector.tensor_scalar(q[:].bitcast(I32), q[:].bitcast(I32),
                                        EXP_MASK, None, A.bitwise_and)
                nc.vector.tensor_tensor(wT[:, k, :], q[:], wint[:], A.mult)

            # ---- Phase B: stream bf16 x groups, matmul, bias-add drain, store
            for tg in range(ntg if do_mm else 0):
                xt = bp.tile([128, nk, TG], BF16, tag="xt", bufs=2)
                nc.sync.dma_start(xt[:], xt_d[:, tg, :, :])
                for ts in range(TG // 128):
                    tt = tg * (TG // 128) + ts
                    ps = pps.tile([128, osh], F32, tag="ps")
                    for k in range(nk):
                        nc.tensor.matmul(ps[:], xt[:, k, ts * 128:(ts + 1) * 128], wT[:, k, :],
                                         start=(k == 0), stop=(k == nk - 1))
                    os_t = bp.tile([128, osh], F32, tag="os", bufs=3)
                    if do_bias:
                        nc.vector.tensor_tensor(os_t[:], ps[:], bias_b[:], A.add)
                    else:
                        nc.scalar.copy(os_t[:], ps[:])
                    nc.scalar.dma_start(out_d[tt * 128:(tt + 1) * 128, :], os_t[:])

    nc.compile()
    return nc


_CACHE = {}


def _get(cfg_key):
    if cfg_key not in _CACHE:
        _CACHE[cfg_key] = build(**dict(cfg_key))
    return _CACHE[cfg_key]


def make_in_maps(x2d, ori, delta, zp, aux, laA, laB, bias, n_cores, osh):
    import ml_dtypes
    tok, i = x2d.shape
    nk = i // 128
    ntg = tok // TG
    # xtr[p, tg, k, t] = x[tg*TG + t, k*128 + p]
    xtr = np.ascontiguousarray(
        x2d.astype(ml_dtypes.bfloat16).reshape(ntg, TG, nk, 128).transpose(3, 0, 2, 1))
    in_maps = []
    for c in range(n_cores):
        sl = slice(c * osh, (c + 1) * osh)
        # [osh, i] -> [i, osh] -> [p, k, o] with i = k*128 + p
        oriT = ori[sl].T.reshape(nk, 128, osh).transpose(1, 0, 2)
        auxT = aux[sl].T.reshape(nk, 128, osh).transpose(1, 0, 2)
        in_maps.append({
            "xtr": xtr,
            "ori": np.ascontiguousarray(oriT).astype(np.uint8),
            "aux": np.ascontiguousarray(auxT).astype(np.float16),
            "zp": np.ascontiguousarray(zp[sl]).reshape(1, osh),
            "dl": np.ascontiguousarray(delta[sl]).reshape(1, osh),
            "bt": np.ascontiguousarray(laB[sl].T),
            "la": laA,
            "bias": np.ascontiguousarray(bias[sl]).reshape(1, osh),
        })
    return in_maps


def kernel(x, ori_weight_round, weight_quant_delta, weight_quant_zero_point,
           aux_R, loraA_w, loraB_w, bias, _trace=False):
    cfg = FULL
    n_cores, osh = cfg["n_cores"], cfg["osh"]
    x2d = np.ascontiguousarray(np.asarray(x, dtype=np.float32).reshape(cfg["tok"], cfg["i"]))
    nc = _get(tuple(sorted(cfg.items())))
    in_maps = make_in_maps(
        x2d,
        np.asarray(ori_weight_round, np.float32),
        np.asarray(weight_quant_delta, np.float32),
        np.asarray(weight_quant_zero_point, np.float32),
        np.asarray(aux_R, np.float32),
        np.asarray(loraA_w, np.float32),
        np.asarray(loraB_w, np.float32),
        np.asarray(bias, np.float32),
        n_cores, osh)
    res = run_bass_kernel_spmd(nc, in_maps, core_ids=list(range(n_cores)), trace=_trace)
    out = np.concatenate([res.results[c]["out"] for c in range(n_cores)], axis=1)
    out = out.reshape(B_, S_, O_)
    if _trace:
        return out, res
    return out


# revision 15
# speedup vs baseline: 1.7637x; 1.4205x over previous
"""IntLoRA-SHIFT fused kernel for Trainium2 (8 NeuronCores, tensor-parallel on out_features).

Math (per reference):
    w_int  = ori_weight_round - zero_point                    [O, I]
    lora   = (aux_R + loraB @ loraA) / where(w_int==0, 1, w_int)
    wu     = delta + lora
    weight = sign(wu) * 2^round(log2|wu|) * w_int
    out    = x @ weight.T + bias

Kernel strategy per core (O sharded 8 x 512):
  Everything lives in [i(partition), o(free)] layout so weight prep emits the
  k-tile-transposed weight wT directly (no on-device transposes).  The host
  pre-transposes ori (uint8) and aux (fp16) per core; per-o params
  (zero_point/delta/bias) are partition-broadcast once into [128, osh] tiles.

  Phase A (per i-chunk k, tiles [128, 512]):
    PE:  ba = (loraB@loraA)^T slice  (K=4 fp32 matmul into PSUM)
    GpS: wint = ori - zp_b ;  den = wint + (wint==0)
    DVE: rcp ~= 1/den (fast, 18-bit) ; num = aux + ba ; num *= rcp
    GpS: num += dl_b
    DVE: q = bitcast((bits(num) + 0x4AFB0C) & 0xFF800000)   == sign*2^round(log2|.|)
    DVE: wT[:,k,:] = q * wint  (exact in bf16: |wint|<=255 has 8 significand bits)
  Work is split DVE(4-5 passes)/GpSimd(3 passes) so the two engines pipeline
  across chunks; PE+DMA overlap underneath.

  Phase B: host-transposed bf16 x streamed as [128, nk, 512] groups (contig
  32KB/partition DMAs), 32 bf16 matmuls per 128-token tile accumulate in PSUM,
  drained by DVE adds (+bias broadcast tile) and stored with HWDGE DMAs.
  No fp32 matmuls in the loop: PE stays in bf16 mode at the roofline.
"""
import os
import sys

for _p in ("/root/.axon_site", "/root/.axon_site/_ro/trn_rl_repo", "/root/.axon_site/_ro/pypackages", "/opt/trn_rl_repo"):
    if os.path.isdir(_p) and _p not in sys.path:
        sys.path.append(_p)

import numpy as np

import concourse.bacc as bacc
import concourse.mybir as mybir
import concourse.tile as tile
from concourse.bass_utils import run_bass_kernel_spmd

A = mybir.AluOpType
F32 = mybir.dt.float32
F16 = mybir.dt.float16
BF16 = mybir.dt.bfloat16
I32 = mybir.dt.int32
U8 = mybir.dt.uint8

C_ROUND = 0x004AFB0C                                   # carry threshold: mantissa >= sqrt(2)
EXP_MASK = int(np.uint32(0xFF800000).view(np.int32))   # sign+exponent mask

# full problem config
FULL = dict(tok=8192, i=4096, osh=512, r=4, n_cores=8)
B_, S_, O_ = 4, 2048, 4096
TG = 512


def build(tok, i, osh, r, n_cores, reps=1, do_prep=True, do_mm=True, do_bias=True):
    """Build + compile the per-core kernel (SPMD: same program, sharded data).

    reps>1 wraps the whole body in a hardware For_i loop (for benchmarking:
    one dispatch executes the body `reps` times)."""
    nk = i // 128          # contraction k-tiles
    ntg = tok // TG        # 512-token groups

    nc = bacc.Bacc("TRN2", target_bir_lowering=False, debug=False,
                   enable_asserts=False, num_devices=n_cores)
    xt_d = nc.dram_tensor("xtr", [128, ntg, nk, TG], BF16, kind="ExternalInput").ap()
    ori_d = nc.dram_tensor("ori", [128, nk, osh], U8, kind="ExternalInput").ap()
    aux_d = nc.dram_tensor("aux", [128, nk, osh], F32, kind="ExternalInput").ap()
    zp_d = nc.dram_tensor("zp", [1, osh], F32, kind="ExternalInput").ap()
    dl_d = nc.dram_tensor("dl", [1, osh], F32, kind="ExternalInput").ap()
    bt_d = nc.dram_tensor("bt", [r, osh], F32, kind="ExternalInput").ap()   # loraB shard, pre-transposed
    la_d = nc.dram_tensor("la", [r, i], F32, kind="ExternalInput").ap()     # loraA
    bias_d = nc.dram_tensor("bias", [1, osh], F32, kind="ExternalInput").ap()
    out_d = nc.dram_tensor("out", [tok, osh], F32, kind="ExternalOutput").ap()

    import contextlib

    with tile.TileContext(nc) as tc:
        with tc.tile_pool(name="const", bufs=1) as cp, \
             tc.tile_pool(name="wres", bufs=1) as wp, \
             tc.tile_pool(name="prep", bufs=2) as pr, \
             tc.tile_pool(name="bpool", bufs=1) as bp, \
             tc.tile_pool(name="pba", bufs=2, space="PSUM") as pba, \
             tc.tile_pool(name="pps", bufs=4, space="PSUM") as pps, \
             (tc.For_i(0, reps, 1) if reps > 1 else contextlib.nullcontext()):

            # ---- constants + broadcasts
            la_sb = cp.tile([r, i], F32)
            nc.sync.dma_start(la_sb[:], la_d[:])
            bt_sb = cp.tile([r, osh], F32)
            nc.sync.dma_start(bt_sb[:], bt_d[:])
            zp_row = cp.tile([1, osh], F32)
            nc.sync.dma_start(zp_row[:], zp_d[:])
            dl_row = cp.tile([1, osh], F32)
            nc.sync.dma_start(dl_row[:], dl_d[:])
            bias_row = cp.tile([1, osh], F32)
            nc.sync.dma_start(bias_row[:], bias_d[:])

            zp_b = cp.tile([128, osh], F32)
            nc.gpsimd.partition_broadcast(zp_b[:], zp_row[:])
            dl_b = cp.tile([128, osh], F32)
            nc.gpsimd.partition_broadcast(dl_b[:], dl_row[:])
            bias_b = cp.tile([128, osh], F32)
            nc.gpsimd.partition_broadcast(bias_b[:], bias_row[:])
            # whole ori shard resident in SBUF (16KB per partition)
            ori_all = cp.tile([128, nk, osh], U8)
            nc.sync.dma_start(ori_all[:], ori_d[:])

            # resident transposed weight: [i(128), k, o]
            wT = wp.tile([128, nk, osh], BF16)
            if not do_prep:
                nc.vector.memset(wT[:, 0, 0:1], 0.0)   # touch so reads are legal

            # ---- Phase A: weight prep per i-chunk (7 DVE passes; ACT+PE+DMA
            # handle the BA term).  den = (ori + 1e-4) - zp stands in for
            # w_int: the epsilon survives the add (> half-ulp of 255), so
            # den != 0 even when w_int == 0 (keeps recip finite), while
            # q * den still rounds to the exact bf16 weight for w_int != 0.
            for k in range(nk if do_prep else 0):
                ba = pba.tile([128, osh], F32, tag="ba")
                nc.tensor.matmul(ba[:], la_sb[:, k * 128:(k + 1) * 128], bt_sb[:],
                                 start=True, stop=True)
                aux_t = pr.tile([128, osh], F32, tag="aux")
                nc.sync.dma_start(aux_t[:], aux_d[:, k, :])

                den = pr.tile([128, osh], F32, tag="den")
                nc.vector.scalar_tensor_tensor(den[:], ori_all[:, k, :], 1e-4,
                                               zp_b[:], A.add, A.subtract)
                rcp = pr.tile([128, osh], F32, tag="rcp")
                nc.vector.reciprocal_approx_fast(rcp[:], den[:])
                num = pr.tile([128, osh], F32, tag="num")
                nc.vector.tensor_tensor(num[:], aux_t[:], ba[:], A.add)
                nc.vector.tensor_tensor(num[:], num[:], rcp[:], A.mult)
                nc.vector.tensor_tensor(num[:], num[:], dl_b[:], A.add)
                q = pr.tile([128, osh], F32, tag="q")
                nc.vector.tensor_scalar(q[:].bitcast(I32), num[:].bitcast(I32),
                                        C_ROUND, None, A.add)
                nc.vector.tensor_scalar(q[:].bitcast(I32), q[:].bitcast(I32),
                                        EXP_MASK, None, A.bitwise_and)
                nc.vector.tensor_tensor(wT[:, k, :], q[:], den[:], A.mult)

            # ---- Phase B: stream bf16 x groups, matmul, bias-add drain, store
            for tg in range(ntg if do_mm else 0):
                xt = bp.tile([128, nk, TG], BF16, tag="xt", bufs=2)
                nc.sync.dma_start(xt[:], xt_d[:, tg, :, :])
                for ts in range(TG // 128):
                    tt = tg * (TG // 128) + ts
                    ps = pps.tile([128, osh], F32, tag="ps")
                    for k in range(nk):
                        nc.tensor.matmul(ps[:], xt[:, k, ts * 128:(ts + 1) * 128], wT[:, k, :],
                                         start=(k == 0), stop=(k == nk - 1))
                    os_t = bp.tile([128, osh], F32, tag="os", bufs=3)
                    if do_bias:
                        nc.vector.tensor_tensor(os_t[:], ps[:], bias_b[:], A.add)
                    else:
                        nc.scalar.copy(os_t[:], ps[:])
                    nc.scalar.dma_start(out_d[tt * 128:(tt + 1) * 128, :], os_t[:])

    nc.compile()
    return nc


_CACHE = {}


def _get(cfg_key):
    if cfg_key not in _CACHE:
        _CACHE[cfg_key] = build(**dict(cfg_key))
    return _CACHE[cfg_key]


def make_in_maps(x2d, ori, delta, zp, aux, laA, laB, bias, n_cores, osh):
    import ml_dtypes
    tok, i = x2d.shape
    nk = i // 128
    ntg = tok // TG
    # xtr[p, tg, k, t] = x[tg*TG + t, k*128 + p]
    xtr = np.ascontiguousarray(
        x2d.astype(ml_dtypes.bfloat16).reshape(ntg, TG, nk, 128).transpose(3, 0, 2, 1))
    in_maps = []
    for c in range(n_cores):
        sl = slice(c * osh, (c + 1) * osh)
        # [osh, i] -> [i, osh] -> [p, k, o] with i = k*128 + p
        oriT = ori[sl].T.reshape(nk, 128, osh).transpose(1, 0, 2)
        auxT = aux[sl].T.reshape(nk, 128, osh).transpose(1, 0, 2)
        in_maps.append({
            "xtr": xtr,
            "ori": np.ascontiguousarray(oriT).astype(np.uint8),
            "aux": np.ascontiguousarray(auxT).astype(np.float32),
            "zp": np.ascontiguousarray(zp[sl]).reshape(1, osh),
            "dl": np.ascontiguousarray(delta[sl]).reshape(1, osh),
            "bt": np.ascontiguousarray(laB[sl].T),
            "la": laA,
            "bias": np.ascontiguousarray(bias[sl]).reshape(1, osh),
        })
    return in_maps


def kernel(x, ori_weight_round, weight_quant_delta, weight_quant_zero_point,
           aux_R, loraA_w, loraB_w, bias, _trace=False):
    cfg = FULL
    n_cores, osh = cfg["n_cores"], cfg["osh"]
    x2d = np.ascontiguousarray(np.asarray(x, dtype=np.float32).reshape(cfg["tok"], cfg["i"]))
    nc = _get(tuple(sorted(cfg.items())))
    in_maps = make_in_maps(
        x2d,
        np.asarray(ori_weight_round, np.float32),
        np.asarray(weight_quant_delta, np.float32),
        np.asarray(weight_quant_zero_point, np.float32),
        np.asarray(aux_R, np.float32),
        np.asarray(loraA_w, np.float32),
        np.asarray(loraB_w, np.float32),
        np.asarray(bias, np.float32),
        n_cores, osh)
    res = run_bass_kernel_spmd(nc, in_maps, core_ids=list(range(n_cores)), trace=_trace)
    out = np.concatenate([res.results[c]["out"] for c in range(n_cores)], axis=1)
    out = out.reshape(B_, S_, O_)
    if _trace:
        return out, res
    return out


# revision 22
# speedup vs baseline: 1.9115x; 1.0838x over previous
"""IntLoRA-SHIFT fused kernel for Trainium2 (8 NeuronCores, tensor-parallel on out_features).

Math (per reference):
    w_int  = ori_weight_round - zero_point                    [O, I]
    lora   = (aux_R + loraB @ loraA) / where(w_int==0, 1, w_int)
    wu     = delta + lora
    weight = sign(wu) * 2^round(log2|wu|) * w_int
    out    = x @ weight.T + bias

Kernel strategy per core (O sharded 8 x 512):
  Everything lives in [i(partition), o(free)] layout so weight prep emits the
  k-tile-transposed weight wT directly (no on-device transposes).  The host
  pre-transposes ori (uint8) and aux (fp16) per core; per-o params
  (zero_point/delta/bias) are partition-broadcast once into [128, osh] tiles.

  Phase A (per i-chunk k, tiles [128, 512]):
    PE:  ba = (loraB@loraA)^T slice  (K=4 fp32 matmul into PSUM)
    GpS: wint = ori - zp_b ;  den = wint + (wint==0)
    DVE: rcp ~= 1/den (fast, 18-bit) ; num = aux + ba ; num *= rcp
    GpS: num += dl_b
    DVE: q = bitcast((bits(num) + 0x4AFB0C) & 0xFF800000)   == sign*2^round(log2|.|)
    DVE: wT[:,k,:] = q * wint  (exact in bf16: |wint|<=255 has 8 significand bits)
  Work is split DVE(4-5 passes)/GpSimd(3 passes) so the two engines pipeline
  across chunks; PE+DMA overlap underneath.

  Phase B: host-transposed bf16 x streamed as [128, nk, 512] groups (contig
  32KB/partition DMAs), 32 bf16 matmuls per 128-token tile accumulate in PSUM,
  drained by DVE adds (+bias broadcast tile) and stored with HWDGE DMAs.
  No fp32 matmuls in the loop: PE stays in bf16 mode at the roofline.
"""
import os
import sys

for _p in ("/root/.axon_site", "/root/.axon_site/_ro/trn_rl_repo", "/root/.axon_site/_ro/pypackages", "/opt/trn_rl_repo"):
    if os.path.isdir(_p) and _p not in sys.path:
        sys.path.append(_p)

import numpy as np

import concourse.bacc as bacc
import concourse.mybir as mybir
import concourse.tile as tile
from concourse.bass_utils import run_bass_kernel_spmd

A = mybir.AluOpType
F32 = mybir.dt.float32
F16 = mybir.dt.float16
BF16 = mybir.dt.bfloat16
I32 = mybir.dt.int32
U8 = mybir.dt.uint8

SQRT2 = float(np.float32(np.sqrt(2.0)))                # round-to-nearest-log2 threshold

# full problem config
FULL = dict(tok=8192, i=4096, osh=512, r=4, n_cores=8)
B_, S_, O_ = 4, 2048, 4096
TG = 512


def build(tok, i, osh, r, n_cores, reps=1, do_prep=True, do_mm=True, do_bias=True):
    """Build + compile the per-core kernel (SPMD: same program, sharded data).

    reps>1 wraps the whole body in a hardware For_i loop (for benchmarking:
    one dispatch executes the body `reps` times)."""
    nk = i // 128          # contraction k-tiles
    ntg = tok // TG        # 512-token groups

    nc = bacc.Bacc("TRN2", target_bir_lowering=False, debug=False,
                   enable_asserts=False, num_devices=n_cores)
    xt_d = nc.dram_tensor("xtr", [128, ntg, nk, TG], BF16, kind="ExternalInput").ap()
    ori_d = nc.dram_tensor("ori", [128, nk, osh], U8, kind="ExternalInput").ap()
    aux_d = nc.dram_tensor("aux", [128, nk, osh], F32, kind="ExternalInput").ap()
    zp_d = nc.dram_tensor("zp", [1, osh], F32, kind="ExternalInput").ap()
    dl_d = nc.dram_tensor("dl", [1, osh], F32, kind="ExternalInput").ap()
    bt_d = nc.dram_tensor("bt", [r, osh], BF16, kind="ExternalInput").ap()  # loraB shard, pre-transposed
    la_d = nc.dram_tensor("la", [r, i], BF16, kind="ExternalInput").ap()    # loraA
    bias_d = nc.dram_tensor("bias", [1, osh], F32, kind="ExternalInput").ap()
    out_d = nc.dram_tensor("out", [tok, osh], F32, kind="ExternalOutput").ap()

    import contextlib

    with tile.TileContext(nc) as tc:
        with tc.tile_pool(name="const", bufs=1) as cp, \
             tc.tile_pool(name="wres", bufs=1) as wp, \
             tc.tile_pool(name="prep", bufs=2) as pr, \
             tc.tile_pool(name="bpool", bufs=1) as bp, \
             tc.tile_pool(name="pba", bufs=2, space="PSUM") as pba, \
             tc.tile_pool(name="pps", bufs=4, space="PSUM") as pps, \
             (tc.For_i(0, reps, 1) if reps > 1 else contextlib.nullcontext()):

            # ---- constants + broadcasts
            la_sb = cp.tile([r, i], BF16)
            nc.sync.dma_start(la_sb[:], la_d[:])
            bt_sb = cp.tile([r, osh], BF16)
            nc.sync.dma_start(bt_sb[:], bt_d[:])
            zp_row = cp.tile([1, osh], F32)
            nc.sync.dma_start(zp_row[:], zp_d[:])
            dl_row = cp.tile([1, osh], F32)
            nc.sync.dma_start(dl_row[:], dl_d[:])
            bias_row = cp.tile([1, osh], F32)
            nc.sync.dma_start(bias_row[:], bias_d[:])

            zp_b = cp.tile([128, osh], F32)
            nc.gpsimd.partition_broadcast(zp_b[:], zp_row[:])
            dl_b = cp.tile([128, osh], F32)
            nc.gpsimd.partition_broadcast(dl_b[:], dl_row[:])
            nc.vector.tensor_scalar(dl_b[:], dl_b[:], SQRT2, None, A.mult)
            bias_b = cp.tile([128, osh], F32)
            nc.gpsimd.partition_broadcast(bias_b[:], bias_row[:])
            # resident transposed weight: [i(128), k, o]
            wT = wp.tile([128, nk, osh], BF16)
            if not do_prep:
                nc.vector.memset(wT[:, 0, 0:1], 0.0)   # touch so reads are legal

            # ---- Phase A: weight prep per i-chunk (7 DVE passes; PE+DMA feed
            # the BA term).  den = (ori + 1e-4) - zp stands in for w_int: the
            # epsilon survives the add (> half-ulp of 255), so den != 0 even
            # when w_int == 0 (keeps recip finite), while q * den still
            # rounds to the exact bf16 weight for w_int != 0.
            # sign*2^round(log2|wu|) is computed as exponent-floor of
            # wu*sqrt2: the sqrt2 mult is fused with the delta add (delta
            # pre-scaled by sqrt2) and the mantissa clear is a fused
            # logical shift-right/left pair.
            for k in range(nk if do_prep else 0):
                ba = pba.tile([128, osh], F32, tag="ba")
                nc.tensor.matmul(ba[:], la_sb[:, k * 128:(k + 1) * 128], bt_sb[:],
                                 start=True, stop=True)
                aux_t = pr.tile([128, osh], F32, tag="aux")
                nc.sync.dma_start(aux_t[:], aux_d[:, k, :])
                ori_t = pr.tile([128, osh], U8, tag="ori")
                nc.sync.dma_start(ori_t[:], ori_d[:, k, :])

                den = pr.tile([128, osh], F32, tag="den")
                nc.vector.scalar_tensor_tensor(den[:], ori_t[:], 1e-4,
                                               zp_b[:], A.add, A.subtract)
                rcp = pr.tile([128, osh], F32, tag="rcp")
                nc.vector.reciprocal_approx_fast(rcp[:], den[:])
                num = pr.tile([128, osh], F32, tag="num")
                nc.vector.tensor_tensor(num[:], aux_t[:], ba[:], A.add)
                nc.vector.tensor_tensor(num[:], num[:], rcp[:], A.mult)
                nc.vector.scalar_tensor_tensor(num[:], num[:], SQRT2, dl_b[:],
                                               A.mult, A.add)
                q = pr.tile([128, osh], F32, tag="q")
                nc.vector.tensor_scalar(q[:].bitcast(I32), num[:].bitcast(I32),
                                        23, 23, A.logical_shift_right,
                                        A.logical_shift_left)
                nc.vector.tensor_tensor(wT[:, k, :], q[:], den[:], A.mult)

            # ---- Phase B: stream bf16 x groups, matmul, bias-add drain, store
            for tg in range(ntg if do_mm else 0):
                xt = bp.tile([128, nk, TG], BF16, tag="xt", bufs=2)
                nc.sync.dma_start(xt[:], xt_d[:, tg, :, :])
                for ts in range(TG // 128):
                    tt = tg * (TG // 128) + ts
                    ps = pps.tile([128, osh], F32, tag="ps")
                    for k in range(nk):
                        nc.tensor.matmul(ps[:], xt[:, k, ts * 128:(ts + 1) * 128], wT[:, k, :],
                                         start=(k == 0), stop=(k == nk - 1))
                    os_t = bp.tile([128, osh], F32, tag="os", bufs=3)
                    if do_bias:
                        nc.vector.tensor_tensor(os_t[:], ps[:], bias_b[:], A.add)
                    else:
                        nc.scalar.copy(os_t[:], ps[:])
                    nc.scalar.dma_start(out_d[tt * 128:(tt + 1) * 128, :], os_t[:])

    nc.compile()
    return nc


_CACHE = {}


def _get(cfg_key):
    if cfg_key not in _CACHE:
        _CACHE[cfg_key] = build(**dict(cfg_key))
    return _CACHE[cfg_key]


def make_in_maps(x2d, ori, delta, zp, aux, laA, laB, bias, n_cores, osh):
    import ml_dtypes
    tok, i = x2d.shape
    nk = i // 128
    ntg = tok // TG
    # xtr[p, tg, k, t] = x[tg*TG + t, k*128 + p]
    xtr = np.ascontiguousarray(
        x2d.astype(ml_dtypes.bfloat16).reshape(ntg, TG, nk, 128).transpose(3, 0, 2, 1))
    in_maps = []
    for c in range(n_cores):
        sl = slice(c * osh, (c + 1) * osh)
        # [osh, i] -> [i, osh] -> [p, k, o] with i = k*128 + p
        oriT = ori[sl].T.reshape(nk, 128, osh).transpose(1, 0, 2)
        auxT = aux[sl].T.reshape(nk, 128, osh).transpose(1, 0, 2)
        in_maps.append({
            "xtr": xtr,
            "ori": np.ascontiguousarray(oriT).astype(np.uint8),
            "aux": np.ascontiguousarray(auxT).astype(np.float32),
            "zp": np.ascontiguousarray(zp[sl]).reshape(1, osh),
            "dl": np.ascontiguousarray(delta[sl]).reshape(1, osh),
            "bt": np.ascontiguousarray(laB[sl].T).astype(ml_dtypes.bfloat16),
            "la": laA.astype(ml_dtypes.bfloat16),
            "bias": np.ascontiguousarray(bias[sl]).reshape(1, osh),
        })
    return in_maps


def kernel(x, ori_weight_round, weight_quant_delta, weight_quant_zero_point,
           aux_R, loraA_w, loraB_w, bias, _trace=False):
    cfg = FULL
    n_cores, osh = cfg["n_cores"], cfg["osh"]
    x2d = np.ascontiguousarray(np.asarray(x, dtype=np.float32).reshape(cfg["tok"], cfg["i"]))
    nc = _get(tuple(sorted(cfg.items())))
    in_maps = make_in_maps(
        x2d,
        np.asarray(ori_weight_round, np.float32),
        np.asarray(weight_quant_delta, np.float32),
        np.asarray(weight_quant_zero_point, np.float32),
        np.asarray(aux_R, np.float32),
        np.asarray(loraA_w, np.float32),
        np.asarray(loraB_w, np.float32),
        np.asarray(bias, np.float32),
        n_cores, osh)
    res = run_bass_kernel_spmd(nc, in_maps, core_ids=list(range(n_cores)), trace=_trace)
    out = np.concatenate([res.results[c]["out"] for c in range(n_cores)], axis=1)
    out = out.reshape(B_, S_, O_)
    if _trace:
        return out, res
    return out
